# revision 31
# baseline (speedup 1.0000x reference)
"""DecayMaskedMultiHeadAttention on 8 trn2 NeuronCores (Bass/Tile SPMD).

Model: B=4, N=1024, DIM=1024, 16 heads x head_dim 64.
  q/k/v = x @ W.T + b ; scores = (q_h k_h^T)/8 * decaymask_h ;
  out = softmax(scores) v_h ; y = concat_h(out) @ wo.T + bo

Sharding (8 cores): 4 head-groups x 2 batch-groups.
  core c: head group g = c // 2 (heads 4g..4g+3), batch group p = c % 2
  (batches 2p, 2p+1). Each core computes a partial y for its 2 batches;
  host sums the 4 partials per batch group and adds bo + bv @ wo.T
  (attention rows sum to 1 so bv passes through exactly).

All-f16 matmul paths (fp8 fails the 2e-2 rel-err gate on the softmax
path), optimized around the DVE mask-multiply which paces the attention
region (~1.22us per [128,1024] kc chunk):
  - wide ops: scores PSUM tile [128,1024] spanning 2 banks (one DVE
    mask-mul per kc), exp over [128, 2, 1024] (FD 2048 per ACT op).
  - reciprocal_approx_fast for softmax denominators (5x vs DVE
    reciprocal; needs SBUF input at partition 0, so ACT first copies
    the PSUM denominator row down).
  - batched, host-pre-permuted DMA layouts: every transfer is
    partition-contiguous and issued as one DMA_DIRECT2D (~600ns of
    HWDGE ring time each), inputs ordered by need (x0, w, mask h0,
    x1, ...); outputs written as [128, 2, 1024] tci-pair blocks.
  - schedule: all 4 heads of batch 0 first (b1 projections ride as
    per-kc PE filler pieces), then batch 1 (outproj b0 as fillers);
    all 4 mask heads stay resident in SBUF so each is DMA'd once;
    PE pre-warm matmuls lift the HAM clock gate during startup DMA;
    normalize of head h and its last attn@V pair defer into head
    h+1's first kc slots (pipelined head transitions).
  - output f16 (host accumulates partials in f32).
"""

import numpy as np
import ml_dtypes

DIM = 1024
H = 16
HD = 64
B = 4
N = 1024
NCORES = 8
HPC = 4            # heads per core
BPC = 2            # batches per core
NSTACK = 2         # 2-head stacks per core
VBLK = 65          # v block per head: 64 data + ones col(64)
VROW = HPC * VBLK  # v columns per 128-token chunk
KC = DIM // 128    # 8 contraction chunks over D
TC = N // 128      # 8 token chunks
QH = N // 512      # 2 q halves (PSUM bank = 512 fp32)

_PROGRAM = None
LAST_RESULTS = None  # BassKernelResults from the most recent run (for test.py)


def _build_program():
    import concourse.mybir as mybir
    import concourse.tile as tile
    from concourse import bacc

    f32 = mybir.dt.float32
    f16 = mybir.dt.float16
    AF = mybir.ActivationFunctionType

    nc = bacc.Bacc(
        "TRN2",
        target_bir_lowering=False,
        debug=False,
        num_devices=NCORES,
    )

    xT = nc.dram_tensor("xT", [BPC, 128, KC, N], f16, kind="ExternalInput").ap()
    maskT = nc.dram_tensor("maskT", [HPC, 128, TC, N], f16, kind="ExternalInput").ap()
    wqT = nc.dram_tensor("wqT", [128, KC, HPC * HD], f16, kind="ExternalInput").ap()
    wkT = nc.dram_tensor("wkT", [128, KC, HPC * HD], f16, kind="ExternalInput").ap()
    wvT = nc.dram_tensor("wvT", [128, KC, HPC * HD], f16, kind="ExternalInput").ap()
    woT = nc.dram_tensor("woT", [HPC * HD, DIM], f16, kind="ExternalInput").ap()
    bqd = nc.dram_tensor("bq", [128, NSTACK], f32, kind="ExternalInput").ap()
    bkd = nc.dram_tensor("bk", [128, NSTACK], f32, kind="ExternalInput").ap()
    outp = nc.dram_tensor("outp", [BPC, N, DIM], f16, kind="ExternalOutput").ap()

    with tile.TileContext(nc) as tc:
        with (
            tc.tile_pool(name="w", bufs=1) as wpool,
            tc.tile_pool(name="persist", bufs=1) as persist,
            tc.tile_pool(name="maskp", bufs=8) as maskp,
            tc.tile_pool(name="expm", bufs=4) as expm_p,
            tc.tile_pool(name="expo", bufs=5) as expo_p,
            tc.tile_pool(name="ev", bufs=3) as ev_p,
            tc.tile_pool(name="small", bufs=2) as small_p,
            tc.tile_pool(name="psA", bufs=2, space="PSUM") as psA,
            tc.tile_pool(name="psS", bufs=2, space="PSUM") as psS,
            tc.tile_pool(name="psO", bufs=1, space="PSUM") as psO,
        ):
            # --- persistent weights / activations ---
            wq_t = wpool.tile([128, KC, HPC * HD], f16, tag="wq", name="wq")
            wk_t = wpool.tile([128, KC, HPC * HD], f16, tag="wk", name="wk")
            wv_t = wpool.tile([128, KC, HPC * HD], f16, tag="wv", name="wv")
            wo_t = [wpool.tile([128, DIM], f16, tag=f"wo{s}", name=f"wo{s}")
                    for s in range(NSTACK)]
            bq_t = wpool.tile([128, NSTACK], f32, tag="bq", name="bq")
            bk_t = wpool.tile([128, NSTACK], f32, tag="bk", name="bk")

            qT = {}
            kT = {}
            ao = {}
            for b in range(BPC):
                for s in range(NSTACK):
                    qT[(b, s)] = persist.tile([128, N], f16, tag=f"qT{b}{s}", name=f"qT{b}{s}")
                    kT[(b, s)] = persist.tile([128, N], f16, tag=f"kT{b}{s}", name=f"kT{b}{s}")
                    ao[(b, s)] = persist.tile([128, N], f16, tag=f"ao{b}{s}", name=f"ao{b}{s}")
            xts = {(b, hf): persist.tile([128, KC // 2, N], f16, tag=f"x{b}{hf}", name=f"x{b}{hf}")
                   for b in range(BPC) for hf in range(2)}
            vt = {b: persist.tile([128, TC * VROW], f16, tag=f"v{b}", name=f"v{b}")
                  for b in range(BPC)}
            for b in range(BPC):
                # whole-tile fill; projection evacs overwrite the data
                # columns, leaving ones col(64) + pad cols(65..67) = 1.0
                nc.gpsimd.memset(vt[b][:], 1.0)

            warm = wpool.tile([128, 512], f16, tag="warm", name="warm")
            nc.vector.memset(warm[:], 0.0)

            def prewarm(n_mm):
                """Dummy matmuls to lift the HAM clock gate (4/8 -> 8/8)
                while the startup DMAs stream in."""
                pw = psA.tile([128, 512], f32, tag="big", name="prewarm")
                for i in range(n_mm):
                    nc.tensor.matmul(
                        pw[:],
                        lhsT=warm[:, 0:128],
                        rhs=warm[:],
                        start=(i == 0),
                        stop=(i == n_mm - 1),
                    )

            def load_x(b):
                for hf in range(2):
                    nc.sync.dma_start(
                        xts[(b, hf)][:],
                        xT[b, :, hf * 4:(hf + 1) * 4, :])

            def load_startup():
                """x0 + weights interleaved per kc so the first projection
                matmul can start after a fraction of the DMA."""
                nc.sync.dma_start(xts[(0, 0)][:], xT[0, :, 0:4, :])
                nc.sync.dma_start(wq_t[:], wqT)
                nc.sync.dma_start(xts[(0, 1)][:], xT[0, :, 4:8, :])
                nc.sync.dma_start(wk_t[:], wkT)
                nc.sync.dma_start(wv_t[:], wvT)
                nc.sync.dma_start(bq_t[:], bqd)
                nc.sync.dma_start(bk_t[:], bkd)
                for s in range(NSTACK):
                    nc.sync.dma_start(wo_t[s][:], woT[s * 128:(s + 1) * 128, :])

            def qk_dr(b, wt, bt, dst, s, qh):
                ps = psA.tile([128, 512], f32, tag="big",
                              name=f"pj{b}{s}{qh}{'q' if wt is wq_t else 'k'}")
                for kc in range(KC):
                    nc.tensor.matmul(
                        ps[:],
                        lhsT=wt[:, kc, s * 128:(s + 1) * 128],
                        rhs=xts[(b, kc // 4)][:, kc % 4, qh * 512:(qh + 1) * 512],
                        start=(kc == 0),
                        stop=(kc == KC - 1),
                    )
                nc.scalar.activation(
                    dst[(b, s)][:, qh * 512:(qh + 1) * 512],
                    ps[:],
                    AF.Identity,
                    bias=bt[:, s:s + 1],
                    scale=1.0,
                )

            def v_dr(b, tp):
                """v projection for token chunks 2tp, 2tp+1 (one evac)."""
                ps = psA.tile([128, 512], f32, tag="big", name=f"pjv{b}{tp}")
                for half in range(2):
                    tci = 2 * tp + half
                    for kc in range(KC):
                        nc.tensor.matmul(
                            ps[:, half * 256:(half + 1) * 256],
                            lhsT=xts[(b, kc // 4)][:, kc % 4, tci * 128:(tci + 1) * 128],
                            rhs=wv_t[:, kc, :],
                            start=(kc == 0),
                            stop=(kc == KC - 1),
                        )
                dst = vt[b].rearrange("p (c h e) -> p c h e", h=HPC, e=VBLK)[
                    :, 2 * tp:2 * tp + 2, :, 0:HD]
                src = ps[:].rearrange("p (c h e) -> p c h e", c=2, e=HD)
                nc.scalar.activation(dst, src, AF.Copy)

            def qk_groups(b):
                out = []
                for s in range(NSTACK):
                    for wt, bt, dst in ((wq_t, bq_t, qT), (wk_t, bk_t, kT)):
                        for qh in range(QH):
                            out.append(lambda b=b, wt=wt, bt=bt, dst=dst, s=s, qh=qh:
                                       qk_dr(b, wt, bt, dst, s, qh))
                return out

            def qk_pieces(b, wt, bt, dst, s, qh):
                """qk_dr split into one-matmul pieces (chunk-level DMA deps
                so a piece never stalls the in-order PE queue for long)."""
                st = {}
                tagn = f"pj{b}{s}{qh}{'q' if wt is wq_t else 'k'}"

                def piece(kc):
                    if kc == 0:
                        st['ps'] = psA.tile([128, 512], f32, tag="big", name=tagn)
                    nc.tensor.matmul(
                        st['ps'][:],
                        lhsT=wt[:, kc, s * 128:(s + 1) * 128],
                        rhs=xts[(b, kc // 4)][:, kc % 4, qh * 512:(qh + 1) * 512],
                        start=(kc == 0),
                        stop=(kc == KC - 1),
                    )
                    if kc == KC - 1:
                        nc.scalar.activation(
                            dst[(b, s)][:, qh * 512:(qh + 1) * 512],
                            st['ps'][:],
                            AF.Identity,
                            bias=bt[:, s:s + 1],
                            scale=1.0,
                        )
                return [lambda kc=kc: piece(kc) for kc in range(KC)]

            def v_pieces(b, tp):
                st = {}

                def piece(half, kc):
                    if half == 0 and kc == 0:
                        st['ps'] = psA.tile([128, 512], f32, tag="big", name=f"pjv{b}{tp}")
                    tci = 2 * tp + half
                    nc.tensor.matmul(
                        st['ps'][:, half * 256:(half + 1) * 256],
                        lhsT=xts[(b, kc // 4)][:, kc % 4, tci * 128:(tci + 1) * 128],
                        rhs=wv_t[:, kc, :],
                        start=(kc == 0),
                        stop=(kc == KC - 1),
                    )
                    if half == 1 and kc == KC - 1:
                        dst = vt[b].rearrange("p (c h e) -> p c h e", h=HPC, e=VBLK)[
                            :, 2 * tp:2 * tp + 2, :, 0:HD]
                        srcv = st['ps'][:].rearrange("p (c h e) -> p c h e", c=2, e=HD)
                        nc.scalar.activation(dst, srcv, AF.Copy)
                return [lambda half=half, kc=kc: piece(half, kc)
                        for half in range(2) for kc in range(KC)]

            mask_t = {}

            def preload_mask(h, eng=None):
                for half in range(2):
                    mt = maskp.tile([128, 4, N], f16, tag="mask", name=f"mask{h}_{half}")
                    (eng or nc.sync).dma_start(
                        mt[:], maskT[h, :, half * 4:(half + 1) * 4, :])
                    mask_t[(h, half)] = mt

            v3d = {b: vt[b].rearrange("p (c v) -> p c v", v=VROW) for b in range(BPC)}

            def attn(h, b, pop=None, pending=None, defer=False,
                     defer_finish=False, prev_finish=None):
                """Attention for head h (local), batch b. `pop` is called
                once per kc slot to emit PE filler work. `pending` holds the
                previous head's normalize closures (run at kc 2 and 5 so the
                DVE starts this head's mask-muls first); with defer=True this
                head's normalize is returned instead of emitted."""
                s, hh = h // 2, h % 2
                fin = None
                op = {}
                for qh in range(QH):
                    op[qh] = psO.tile([VBLK, 512], f32, tag=f"ov{qh}", name=f"ov{h}{b}{qh}")
                if b == 0 and h < 2:
                    preload_mask(h + 2)
                for kcp in range(TC // 2):
                    em = expm_p.tile([128, 2, N], f16, tag="expm", name=f"expm{h}{kcp}{b}")
                    eo = expo_p.tile([128, 2, N], f16, tag="expo", name=f"expo{h}{kcp}{b}")
                    for j in range(2):
                        kc = 2 * kcp + j
                        if pop is not None:
                            pop()
                        if pending and kc in (2, 5):
                            pending.pop(0)()
                        sc = psS.tile([128, 1024], f32, tag="sc", name=f"sc{h}{kc}{b}")
                        for qh in range(QH):
                            nc.tensor.matmul(
                                sc[:, qh * 512:(qh + 1) * 512],
                                lhsT=kT[(b, s)][hh * HD:(hh + 1) * HD,
                                                kc * 128:(kc + 1) * 128],
                                rhs=qT[(b, s)][hh * HD:(hh + 1) * HD,
                                               qh * 512:(qh + 1) * 512],
                                start=True,
                                stop=True,
                            )
                        nc.vector.tensor_mul(em[:, j, :], sc[:], mask_t[(h, kc // 4)][:, kc % 4, :])
                        if prev_finish is not None and kcp == 0 and j == 0:
                            prev_finish()
                            prev_finish = None
                    nc.scalar.activation(eo[:], em[:], AF.Exp)

                    def emit_attnv(eo=eo, kcp=kcp):
                        for j in range(2):
                            kc = 2 * kcp + j
                            vblk = v3d[b][:, kc, h * VBLK:(h + 1) * VBLK]
                            for qh in range(QH):
                                nc.tensor.matmul(
                                    op[qh][:],
                                    lhsT=vblk,
                                    rhs=eo[:, j, qh * 512:(qh + 1) * 512],
                                    start=(kc == 0),
                                    stop=(kc == TC - 1),
                                )
                    if defer_finish and kcp == TC // 2 - 1:
                        fin = emit_attnv
                    else:
                        emit_attnv()
                def normalize(qh):
                    # custom-DVE recip needs SBUF input at partition 0 —
                    # ACT copies the PSUM denominator row down first.
                    den = small_p.tile([1, 512], f32, tag="den", name=f"den{h}{b}{qh}")
                    nc.scalar.activation(den[:], op[qh][HD:HD + 1, :], AF.Copy)
                    rc = small_p.tile([1, 512], f32, tag="rec", name=f"rec{h}{b}{qh}")
                    nc.vector.reciprocal_approx_fast(rc[:], den[:])
                    bcs = small_p.tile([HD, 512], f32, tag="bcs", name=f"bcs{h}{b}{qh}")
                    nc.gpsimd.partition_broadcast(bcs[:], rc[:])
                    nc.vector.tensor_mul(
                        ao[(b, s)][hh * HD:(hh + 1) * HD, qh * 512:(qh + 1) * 512],
                        op[qh][0:HD, :],
                        bcs[:],
                    )
                fin = fin if defer_finish else None
                if defer:
                    return [lambda qh=qh: normalize(qh) for qh in range(QH)], fin
                for qh in range(QH):
                    normalize(qh)
                return None, None

            def outproj_group(b, tp, tail=False):
                """Out-projection for token chunks 2tp, 2tp+1 (one 512KB DMA)."""
                ot = ev_p.tile([128, 2, 1024], f16, tag="ot", name=f"ot{b}{tp}")
                for half in range(2):
                    tci = 2 * tp + half
                    for dh in range(QH):
                        po = psA.tile([128, 512], f32, tag="big", name=f"po{b}{tci}{dh}")
                        for s in range(NSTACK):
                            nc.tensor.matmul(
                                po[:],
                                lhsT=ao[(b, s)][:, tci * 128:(tci + 1) * 128],
                                rhs=wo_t[s][:, dh * 512:(dh + 1) * 512],
                                start=(s == 0),
                                stop=(s == NSTACK - 1),
                            )
                        if tail and dh == 0:
                            nc.vector.tensor_copy(
                                ot[:, half, dh * 512:(dh + 1) * 512], po[:])
                        else:
                            nc.scalar.activation(
                                ot[:, half, dh * 512:(dh + 1) * 512], po[:], AF.Copy)
                    if tail:
                        nc.sync.dma_start(
                            outp[b, (2 * tp + half) * 128:
                                 (2 * tp + half + 1) * 128, :],
                            ot[:, half, :])
                if not tail:
                    nc.sync.dma_start(
                        outp[b, 2 * tp * 128:(2 * tp + 2) * 128, :].rearrange(
                            "(t p) n -> p t n", p=128),
                        ot[:])

            # --- emission schedule ---
            prewarm(16)
            load_startup()
            preload_mask(0, eng=nc.scalar)
            for g in qk_groups(0):
                g()
            for tp in range(TC // 2):
                v_dr(0, tp)
            load_x(1)
            preload_mask(1)
            # b0-attention fillers: only what attn(0,1)/attn(1,1) need up
            # front — b1's stack-0 q/k and all of b1's v.  b1's stack-1 q/k
            # (first used by attn(2,1)) rides inside b1's heads 0-1 instead,
            # evening out PE load across the two attention windows.
            fillersA = []
            for wt, bt, dst in ((wq_t, bq_t, qT), (wk_t, bk_t, kT)):
                for qh in range(QH):
                    fillersA += qk_pieces(1, wt, bt, dst, 0, qh)
            for tp in range(TC // 2):
                fillersA += v_pieces(1, tp)

            def make_pop(lst, skip, per_slot):
                state = {'i': 0}

                def pop():
                    i = state['i']
                    state['i'] += 1
                    if i < skip:
                        return
                    for _ in range(per_slot):
                        if lst:
                            lst.pop(0)()
                return pop

            pop0 = make_pop(fillersA, skip=8, per_slot=4)
            pn, fin = attn(0, 0, pop0, defer=True, defer_finish=True)
            pn, fin = attn(1, 0, pop0, pending=pn, defer=True,
                           defer_finish=True, prev_finish=fin)
            pn, fin = attn(2, 0, pop0, pending=pn, defer=True,
                           defer_finish=True, prev_finish=fin)
            attn(3, 0, pop0, pending=pn, prev_finish=fin)
            for f in fillersA:
                f()

            qk1 = []
            for wt, bt, dst in ((wq_t, bq_t, qT), (wk_t, bk_t, kT)):
                for qh in range(QH):
                    qk1 += qk_pieces(1, wt, bt, dst, 1, qh)
            oproj = [lambda tp=tp: outproj_group(0, tp)
                     for tp in range(TC // 2)]

            def make_popB(qk_lst, op_lst):
                state = {'i': 0}

                def pop():
                    i = state['i']
                    state['i'] += 1
                    if i < 16:
                        for _ in range(2):
                            if qk_lst:
                                qk_lst.pop(0)()
                    elif i % 4 == 0 and op_lst:
                        op_lst.pop(0)()
                return pop

            pop1 = make_popB(qk1, oproj)
            pn, fin = attn(0, 1, pop1, defer=True, defer_finish=True)
            pn, fin = attn(1, 1, pop1, pending=pn, defer=True,
                           defer_finish=True, prev_finish=fin)
            for f in qk1:
                f()
            pn, fin = attn(2, 1, pop1, pending=pn, defer=True,
                           defer_finish=True, prev_finish=fin)
            attn(3, 1, pop1, pending=pn, prev_finish=fin)
            for f in oproj:
                f()
            for tp in range(TC // 2):
                outproj_group(1, tp, tail=True)

    nc.compile()
    return nc


def _get_program():
    global _PROGRAM
    if _PROGRAM is None:
        _PROGRAM = _build_program()
    return _PROGRAM


def kernel(x, decaymask, wq, bq, wk, bk, wv, bv, wo, bo):
    from concourse.bass_utils import run_bass_kernel_spmd

    global LAST_RESULTS

    x = np.ascontiguousarray(np.asarray(x, dtype=np.float32))
    decaymask = np.ascontiguousarray(np.asarray(decaymask, dtype=np.float32))
    wq = np.asarray(wq, dtype=np.float32)
    bq = np.asarray(bq, dtype=np.float32)
    wk = np.asarray(wk, dtype=np.float32)
    bk = np.asarray(bk, dtype=np.float32)
    wv = np.asarray(wv, dtype=np.float32)
    bv = np.asarray(bv, dtype=np.float32)
    wo = np.asarray(wo, dtype=np.float32)
    bo = np.asarray(bo, dtype=np.float32)

    nc = _get_program()

    in_maps = []
    for c in range(NCORES):
        g, p = c // 2, c % 2
        rows = slice(g * HPC * HD, (g + 1) * HPC * HD)
        xT_c = np.ascontiguousarray(
            x[p * BPC:(p + 1) * BPC].transpose(0, 2, 1)
            .reshape(BPC, KC, 128, N).transpose(0, 2, 1, 3)
        ).astype(np.float16)  # [BPC, 128, KC, N]
        maskT_c = np.ascontiguousarray(
            decaymask[g * HPC:(g + 1) * HPC].transpose(0, 2, 1)
            .reshape(HPC, TC, 128, N).transpose(0, 2, 1, 3)
        ).astype(np.float16)  # [HPC, 128, kc, q]
        # fold 1/sqrt(HD) = 1/8 (exact) into wq/bq
        def packw(w):
            return np.ascontiguousarray(
                w.reshape(KC, 128, HPC * HD).transpose(1, 0, 2))
        wqT_c = packw(wq[rows, :].T * np.float32(0.125)).astype(np.float16)
        wkT_c = packw(wk[rows, :].T).astype(np.float16)
        wvT_c = packw(wv[rows, :].T).astype(np.float16)
        woT_c = np.ascontiguousarray(wo[:, rows].T).astype(np.float16)
        bq_c = np.ascontiguousarray(
            (bq[rows] * np.float32(0.125)).reshape(NSTACK, 128).T)
        bk_c = np.ascontiguousarray(bk[rows].reshape(NSTACK, 128).T)
        in_maps.append({
            "xT": xT_c,
            "maskT": maskT_c,
            "wqT": wqT_c,
            "wkT": wkT_c,
            "wvT": wvT_c,
            "woT": woT_c,
            "bq": bq_c,
            "bk": bk_c,
        })

    res = run_bass_kernel_spmd(nc, in_maps, list(range(NCORES)))
    LAST_RESULTS = res

    out = np.zeros((B, N, DIM), dtype=np.float32)
    for c in range(NCORES):
        g, p = c // 2, c % 2
        out[p * BPC:(p + 1) * BPC] += res.results[c]["outp"].astype(np.float32)
    out += (bo + bv @ wo.T)[None, None, :]
    return out


# revision 33
# speedup vs baseline: 1.0054x; 1.0054x over previous
"""DecayMaskedMultiHeadAttention on 8 trn2 NeuronCores (Bass/Tile SPMD).

Model: B=4, N=1024, DIM=1024, 16 heads x head_dim 64.
  q/k/v = x @ W.T + b ; scores = (q_h k_h^T)/8 * decaymask_h ;
  out = softmax(scores) v_h ; y = concat_h(out) @ wo.T + bo

Sharding (8 cores): 4 head-groups x 2 batch-groups.
  core c: head group g = c // 2 (heads 4g..4g+3), batch group p = c % 2
  (batches 2p, 2p+1). Each core computes a partial y for its 2 batches;
  host sums the 4 partials per batch group and adds bo + bv @ wo.T
  (attention rows sum to 1 so bv passes through exactly).

All-f16 matmul paths (fp8 fails the 2e-2 rel-err gate on the softmax
path), optimized around the DVE mask-multiply which paces the attention
region (~1.22us per [128,1024] kc chunk):
  - wide ops: scores PSUM tile [128,1024] spanning 2 banks (one DVE
    mask-mul per kc), exp over [128, 2, 1024] (FD 2048 per ACT op).
  - reciprocal_approx_fast for softmax denominators (5x vs DVE
    reciprocal; needs SBUF input at partition 0, so ACT first copies
    the PSUM denominator row down).
  - batched, host-pre-permuted DMA layouts: every transfer is
    partition-contiguous and issued as one DMA_DIRECT2D (~600ns of
    HWDGE ring time each), inputs ordered by need (x0, w, mask h0,
    x1, ...); outputs written as [128, 2, 1024] tci-pair blocks.
  - schedule: all 4 heads of batch 0 first (b1 projections ride as
    per-kc PE filler pieces), then batch 1 (outproj b0 as fillers);
    all 4 mask heads stay resident in SBUF so each is DMA'd once;
    PE pre-warm matmuls lift the HAM clock gate during startup DMA;
    normalize of head h and its last attn@V pair defer into head
    h+1's first kc slots (pipelined head transitions).
  - output f16 (host accumulates partials in f32).
"""

import numpy as np
import ml_dtypes

DIM = 1024
H = 16
HD = 64
B = 4
N = 1024
NCORES = 8
HPC = 4            # heads per core
BPC = 2            # batches per core
NSTACK = 2         # 2-head stacks per core
VBLK = 65          # v block per head: 64 data + ones col(64)
VROW = HPC * VBLK  # v columns per 128-token chunk
KC = DIM // 128    # 8 contraction chunks over D
TC = N // 128      # 8 token chunks
QH = N // 512      # 2 q halves (PSUM bank = 512 fp32)

_PROGRAM = None
LAST_RESULTS = None  # BassKernelResults from the most recent run (for test.py)


def _build_program():
    import concourse.mybir as mybir
    import concourse.tile as tile
    from concourse import bacc

    f32 = mybir.dt.float32
    f16 = mybir.dt.float16
    AF = mybir.ActivationFunctionType

    nc = bacc.Bacc(
        "TRN2",
        target_bir_lowering=False,
        debug=False,
        num_devices=NCORES,
    )

    xT = nc.dram_tensor("xT", [BPC, 128, KC, N], f16, kind="ExternalInput").ap()
    maskT = nc.dram_tensor("maskT", [HPC, 128, TC, N], f16, kind="ExternalInput").ap()
    wqT = nc.dram_tensor("wqT", [128, KC, HPC * HD], f16, kind="ExternalInput").ap()
    wkT = nc.dram_tensor("wkT", [128, KC, HPC * HD], f16, kind="ExternalInput").ap()
    wvT = nc.dram_tensor("wvT", [128, KC, HPC * HD], f16, kind="ExternalInput").ap()
    woT = nc.dram_tensor("woT", [HPC * HD, DIM], f16, kind="ExternalInput").ap()
    bqd = nc.dram_tensor("bq", [128, NSTACK], f32, kind="ExternalInput").ap()
    bkd = nc.dram_tensor("bk", [128, NSTACK], f32, kind="ExternalInput").ap()
    outp = nc.dram_tensor("outp", [BPC, N, DIM], f16, kind="ExternalOutput").ap()

    with tile.TileContext(nc) as tc:
        with (
            tc.tile_pool(name="w", bufs=1) as wpool,
            tc.tile_pool(name="persist", bufs=1) as persist,
            tc.tile_pool(name="maskp", bufs=8) as maskp,
            tc.tile_pool(name="expm", bufs=4) as expm_p,
            tc.tile_pool(name="expo", bufs=4) as expo_p,
            tc.tile_pool(name="ev", bufs=3) as ev_p,
            tc.tile_pool(name="small", bufs=2) as small_p,
            tc.tile_pool(name="psA", bufs=2, space="PSUM") as psA,
            tc.tile_pool(name="psS", bufs=2, space="PSUM") as psS,
            tc.tile_pool(name="psO", bufs=1, space="PSUM") as psO,
        ):
            # --- persistent weights / activations ---
            wq_t = wpool.tile([128, KC, HPC * HD], f16, tag="wq", name="wq")
            wk_t = wpool.tile([128, KC, HPC * HD], f16, tag="wk", name="wk")
            wv_t = wpool.tile([128, KC, HPC * HD], f16, tag="wv", name="wv")
            wo_t = [wpool.tile([128, DIM], f16, tag=f"wo{s}", name=f"wo{s}")
                    for s in range(NSTACK)]
            bq_t = wpool.tile([128, NSTACK], f32, tag="bq", name="bq")
            bk_t = wpool.tile([128, NSTACK], f32, tag="bk", name="bk")

            qT = {}
            kT = {}
            ao = {}
            for b in range(BPC):
                for s in range(NSTACK):
                    qT[(b, s)] = persist.tile([128, N], f16, tag=f"qT{b}{s}", name=f"qT{b}{s}")
                    kT[(b, s)] = persist.tile([128, N], f16, tag=f"kT{b}{s}", name=f"kT{b}{s}")
                    ao[(b, s)] = persist.tile([128, N], f16, tag=f"ao{b}{s}", name=f"ao{b}{s}")
            xts = {(b, hf): persist.tile([128, KC // 2, N], f16, tag=f"x{b}{hf}", name=f"x{b}{hf}")
                   for b in range(BPC) for hf in range(2)}
            vt = {b: persist.tile([128, TC * VROW], f16, tag=f"v{b}", name=f"v{b}")
                  for b in range(BPC)}
            for b in range(BPC):
                # whole-tile fill; projection evacs overwrite the data
                # columns, leaving ones col(64) + pad cols(65..67) = 1.0
                nc.gpsimd.memset(vt[b][:], 1.0)

            warm = wpool.tile([128, 512], f16, tag="warm", name="warm")
            nc.vector.memset(warm[:], 0.0)

            def prewarm(n_mm):
                """Dummy matmuls to lift the HAM clock gate (4/8 -> 8/8)
                while the startup DMAs stream in."""
                pw = psA.tile([128, 512], f32, tag="big", name="prewarm")
                for i in range(n_mm):
                    nc.tensor.matmul(
                        pw[:],
                        lhsT=warm[:, 0:128],
                        rhs=warm[:],
                        start=(i == 0),
                        stop=(i == n_mm - 1),
                    )

            def load_x(b):
                for hf in range(2):
                    nc.sync.dma_start(
                        xts[(b, hf)][:],
                        xT[b, :, hf * 4:(hf + 1) * 4, :])

            def load_startup():
                """x0 + weights interleaved per kc so the first projection
                matmul can start after a fraction of the DMA."""
                nc.sync.dma_start(xts[(0, 0)][:], xT[0, :, 0:4, :])
                nc.sync.dma_start(wq_t[:], wqT)
                nc.sync.dma_start(xts[(0, 1)][:], xT[0, :, 4:8, :])
                nc.sync.dma_start(wk_t[:], wkT)
                nc.sync.dma_start(wv_t[:], wvT)
                nc.sync.dma_start(bq_t[:], bqd)
                nc.sync.dma_start(bk_t[:], bkd)
                for s in range(NSTACK):
                    nc.sync.dma_start(wo_t[s][:], woT[s * 128:(s + 1) * 128, :])

            def qk_dr(b, wt, bt, dst, s, qh):
                ps = psA.tile([128, 512], f32, tag="big",
                              name=f"pj{b}{s}{qh}{'q' if wt is wq_t else 'k'}")
                for kc in range(KC):
                    nc.tensor.matmul(
                        ps[:],
                        lhsT=wt[:, kc, s * 128:(s + 1) * 128],
                        rhs=xts[(b, kc // 4)][:, kc % 4, qh * 512:(qh + 1) * 512],
                        start=(kc == 0),
                        stop=(kc == KC - 1),
                    )
                nc.scalar.activation(
                    dst[(b, s)][:, qh * 512:(qh + 1) * 512],
                    ps[:],
                    AF.Identity,
                    bias=bt[:, s:s + 1],
                    scale=1.0,
                )

            def v_dr(b, tp):
                """v projection for token chunks 2tp, 2tp+1 (one evac)."""
                ps = psA.tile([128, 512], f32, tag="big", name=f"pjv{b}{tp}")
                for half in range(2):
                    tci = 2 * tp + half
                    for kc in range(KC):
                        nc.tensor.matmul(
                            ps[:, half * 256:(half + 1) * 256],
                            lhsT=xts[(b, kc // 4)][:, kc % 4, tci * 128:(tci + 1) * 128],
                            rhs=wv_t[:, kc, :],
                            start=(kc == 0),
                            stop=(kc == KC - 1),
                        )
                dst = vt[b].rearrange("p (c h e) -> p c h e", h=HPC, e=VBLK)[
                    :, 2 * tp:2 * tp + 2, :, 0:HD]
                src = ps[:].rearrange("p (c h e) -> p c h e", c=2, e=HD)
                nc.scalar.activation(dst, src, AF.Copy)

            def qk_groups(b):
                out = []
                for s in range(NSTACK):
                    for wt, bt, dst in ((wq_t, bq_t, qT), (wk_t, bk_t, kT)):
                        for qh in range(QH):
                            out.append(lambda b=b, wt=wt, bt=bt, dst=dst, s=s, qh=qh:
                                       qk_dr(b, wt, bt, dst, s, qh))
                return out

            def qk_pieces(b, wt, bt, dst, s, qh):
                """qk_dr split into one-matmul pieces (chunk-level DMA deps
                so a piece never stalls the in-order PE queue for long)."""
                st = {}
                tagn = f"pj{b}{s}{qh}{'q' if wt is wq_t else 'k'}"

                def piece(kc):
                    if kc == 0:
                        st['ps'] = psA.tile([128, 512], f32, tag="big", name=tagn)
                    nc.tensor.matmul(
                        st['ps'][:],
                        lhsT=wt[:, kc, s * 128:(s + 1) * 128],
                        rhs=xts[(b, kc // 4)][:, kc % 4, qh * 512:(qh + 1) * 512],
                        start=(kc == 0),
                        stop=(kc == KC - 1),
                    )
                    if kc == KC - 1:
                        nc.scalar.activation(
                            dst[(b, s)][:, qh * 512:(qh + 1) * 512],
                            st['ps'][:],
                            AF.Identity,
                            bias=bt[:, s:s + 1],
                            scale=1.0,
                        )
                return [lambda kc=kc: piece(kc) for kc in range(KC)]

            def v_pieces(b, tp):
                st = {}

                def piece(half, kc):
                    if half == 0 and kc == 0:
                        st['ps'] = psA.tile([128, 512], f32, tag="big", name=f"pjv{b}{tp}")
                    tci = 2 * tp + half
                    nc.tensor.matmul(
                        st['ps'][:, half * 256:(half + 1) * 256],
                        lhsT=xts[(b, kc // 4)][:, kc % 4, tci * 128:(tci + 1) * 128],
                        rhs=wv_t[:, kc, :],
                        start=(kc == 0),
                        stop=(kc == KC - 1),
                    )
                    if half == 1 and kc == KC - 1:
                        dst = vt[b].rearrange("p (c h e) -> p c h e", h=HPC, e=VBLK)[
                            :, 2 * tp:2 * tp + 2, :, 0:HD]
                        srcv = st['ps'][:].rearrange("p (c h e) -> p c h e", c=2, e=HD)
                        nc.scalar.activation(dst, srcv, AF.Copy)
                return [lambda half=half, kc=kc: piece(half, kc)
                        for half in range(2) for kc in range(KC)]

            mask_t = {}

            def preload_mask(h, eng=None):
                for half in range(2):
                    mt = maskp.tile([128, 4, N], f16, tag="mask", name=f"mask{h}_{half}")
                    (eng or nc.sync).dma_start(
                        mt[:], maskT[h, :, half * 4:(half + 1) * 4, :])
                    mask_t[(h, half)] = mt

            v3d = {b: vt[b].rearrange("p (c v) -> p c v", v=VROW) for b in range(BPC)}

            def attn(h, b, pop=None, pending=None, defer=False,
                     defer_finish=False, prev_finish=None):
                """Attention for head h (local), batch b. `pop` is called
                once per kc slot to emit PE filler work. `pending` holds the
                previous head's normalize closures (run at kc 2 and 5 so the
                DVE starts this head's mask-muls first); with defer=True this
                head's normalize is returned instead of emitted."""
                s, hh = h // 2, h % 2
                fin = None
                op = {}
                for qh in range(QH):
                    op[qh] = psO.tile([VBLK, 512], f32, tag=f"ov{qh}", name=f"ov{h}{b}{qh}")
                if b == 0 and h < 2:
                    preload_mask(h + 2)
                for kcp in range(TC // 2):
                    em = expm_p.tile([128, 2, N], f16, tag="expm", name=f"expm{h}{kcp}{b}")
                    eo = expo_p.tile([128, 2, N], f16, tag="expo", name=f"expo{h}{kcp}{b}")
                    for j in range(2):
                        kc = 2 * kcp + j
                        if pop is not None:
                            pop()
                        if pending and kc in (2, 5):
                            pending.pop(0)()
                        sc = psS.tile([128, 1024], f32, tag="sc", name=f"sc{h}{kc}{b}")
                        for qh in range(QH):
                            nc.tensor.matmul(
                                sc[:, qh * 512:(qh + 1) * 512],
                                lhsT=kT[(b, s)][hh * HD:(hh + 1) * HD,
                                                kc * 128:(kc + 1) * 128],
                                rhs=qT[(b, s)][hh * HD:(hh + 1) * HD,
                                               qh * 512:(qh + 1) * 512],
                                start=True,
                                stop=True,
                            )
                        nc.vector.tensor_mul(em[:, j, :], sc[:], mask_t[(h, kc // 4)][:, kc % 4, :])
                        if prev_finish is not None and kcp == 0 and j == 0:
                            prev_finish()
                            prev_finish = None
                    nc.scalar.activation(eo[:], em[:], AF.Exp)

                    def emit_attnv(eo=eo, kcp=kcp):
                        for j in range(2):
                            kc = 2 * kcp + j
                            vblk = v3d[b][:, kc, h * VBLK:(h + 1) * VBLK]
                            for qh in range(QH):
                                nc.tensor.matmul(
                                    op[qh][:],
                                    lhsT=vblk,
                                    rhs=eo[:, j, qh * 512:(qh + 1) * 512],
                                    start=(kc == 0),
                                    stop=(kc == TC - 1),
                                )
                    if defer_finish and kcp == TC // 2 - 1:
                        fin = emit_attnv
                    else:
                        emit_attnv()
                def normalize(qh):
                    # custom-DVE recip needs SBUF input at partition 0 —
                    # ACT copies the PSUM denominator row down first.
                    den = small_p.tile([1, 512], f32, tag="den", name=f"den{h}{b}{qh}")
                    nc.scalar.activation(den[:], op[qh][HD:HD + 1, :], AF.Copy)
                    rc = small_p.tile([1, 512], f32, tag="rec", name=f"rec{h}{b}{qh}")
                    nc.vector.reciprocal_approx_fast(rc[:], den[:])
                    bcs = small_p.tile([HD, 512], f32, tag="bcs", name=f"bcs{h}{b}{qh}")
                    nc.gpsimd.partition_broadcast(bcs[:], rc[:])
                    nc.vector.tensor_mul(
                        ao[(b, s)][hh * HD:(hh + 1) * HD, qh * 512:(qh + 1) * 512],
                        op[qh][0:HD, :],
                        bcs[:],
                    )
                fin = fin if defer_finish else None
                if defer:
                    return [lambda qh=qh: normalize(qh) for qh in range(QH)], fin
                for qh in range(QH):
                    normalize(qh)
                return None, None

            def outproj_group(b, tp, tail=False):
                """Out-projection for token chunks 2tp, 2tp+1 (one 512KB DMA)."""
                ot = ev_p.tile([128, 2, 1024], f16, tag="ot", name=f"ot{b}{tp}")
                for half in range(2):
                    tci = 2 * tp + half
                    for dh in range(QH):
                        po = psA.tile([128, 512], f32, tag="big", name=f"po{b}{tci}{dh}")
                        for s in range(NSTACK):
                            nc.tensor.matmul(
                                po[:],
                                lhsT=ao[(b, s)][:, tci * 128:(tci + 1) * 128],
                                rhs=wo_t[s][:, dh * 512:(dh + 1) * 512],
                                start=(s == 0),
                                stop=(s == NSTACK - 1),
                            )
                        if tail and dh == 0:
                            nc.vector.tensor_copy(
                                ot[:, half, dh * 512:(dh + 1) * 512], po[:])
                        else:
                            nc.scalar.activation(
                                ot[:, half, dh * 512:(dh + 1) * 512], po[:], AF.Copy)
                    if tail:
                        nc.sync.dma_start(
                            outp[b, (2 * tp + half) * 128:
                                 (2 * tp + half + 1) * 128, :],
                            ot[:, half, :])
                if not tail:
                    nc.sync.dma_start(
                        outp[b, 2 * tp * 128:(2 * tp + 2) * 128, :].rearrange(
                            "(t p) n -> p t n", p=128),
                        ot[:])

            # --- emission schedule ---
            prewarm(16)
            load_startup()
            preload_mask(0, eng=nc.scalar)
            for g in qk_groups(0):
                g()
            for tp in range(TC // 2):
                v_dr(0, tp)
            load_x(1)
            preload_mask(1)
            # b0-attention fillers: only what attn(0,1)/attn(1,1) need up
            # front — b1's stack-0 q/k and all of b1's v.  b1's stack-1 q/k
            # (first used by attn(2,1)) rides inside b1's heads 0-1 instead,
            # evening out PE load across the two attention windows.
            fillersA = []
            for wt, bt, dst in ((wq_t, bq_t, qT), (wk_t, bk_t, kT)):
                for qh in range(QH):
                    fillersA += qk_pieces(1, wt, bt, dst, 0, qh)
            for tp in range(TC // 2):
                fillersA += v_pieces(1, tp)

            def make_pop(lst, skip, per_slot):
                state = {'i': 0}

                def pop():
                    i = state['i']
                    state['i'] += 1
                    if i < skip:
                        return
                    for _ in range(per_slot):
                        if lst:
                            lst.pop(0)()
                return pop

            pop0 = make_pop(fillersA, skip=8, per_slot=4)
            pn, fin = attn(0, 0, pop0, defer=True, defer_finish=True)
            pn, fin = attn(1, 0, pop0, pending=pn, defer=True,
                           defer_finish=True, prev_finish=fin)
            pn, fin = attn(2, 0, pop0, pending=pn, defer=True,
                           defer_finish=True, prev_finish=fin)
            pn, fin = attn(3, 0, pop0, pending=pn, defer=True,
                           defer_finish=True, prev_finish=fin)
            for f in fillersA:
                f()

            qk1 = []
            for wt, bt, dst in ((wq_t, bq_t, qT), (wk_t, bk_t, kT)):
                for qh in range(QH):
                    qk1 += qk_pieces(1, wt, bt, dst, 1, qh)
            oproj = [lambda tp=tp: outproj_group(0, tp)
                     for tp in range(TC // 2)]

            def make_popB(qk_lst, op_lst):
                state = {'i': 0}

                def pop():
                    i = state['i']
                    state['i'] += 1
                    if i < 16:
                        for _ in range(2):
                            if qk_lst:
                                qk_lst.pop(0)()
                    elif i % 4 == 0 and op_lst:
                        op_lst.pop(0)()
                return pop

            pop1 = make_popB(qk1, oproj)
            pn, fin = attn(0, 1, pop1, pending=pn, defer=True,
                           defer_finish=True, prev_finish=fin)
            pn, fin = attn(1, 1, pop1, pending=pn, defer=True,
                           defer_finish=True, prev_finish=fin)
            for f in qk1:
                f()
            pn, fin = attn(2, 1, pop1, pending=pn, defer=True,
                           defer_finish=True, prev_finish=fin)
            attn(3, 1, pop1, pending=pn, prev_finish=fin)
            for f in oproj:
                f()
            for tp in range(TC // 2):
                outproj_group(1, tp, tail=True)

    nc.compile()
    return nc


def _get_program():
    global _PROGRAM
    if _PROGRAM is None:
        _PROGRAM = _build_program()
    return _PROGRAM


def kernel(x, decaymask, wq, bq, wk, bk, wv, bv, wo, bo):
    from concourse.bass_utils import run_bass_kernel_spmd

    global LAST_RESULTS

    x = np.ascontiguousarray(np.asarray(x, dtype=np.float32))
    decaymask = np.ascontiguousarray(np.asarray(decaymask, dtype=np.float32))
    wq = np.asarray(wq, dtype=np.float32)
    bq = np.asarray(bq, dtype=np.float32)
    wk = np.asarray(wk, dtype=np.float32)
    bk = np.asarray(bk, dtype=np.float32)
    wv = np.asarray(wv, dtype=np.float32)
    bv = np.asarray(bv, dtype=np.float32)
    wo = np.asarray(wo, dtype=np.float32)
    bo = np.asarray(bo, dtype=np.float32)

    nc = _get_program()

    in_maps = []
    for c in range(NCORES):
        g, p = c // 2, c % 2
        rows = slice(g * HPC * HD, (g + 1) * HPC * HD)
        xT_c = np.ascontiguousarray(
            x[p * BPC:(p + 1) * BPC].transpose(0, 2, 1)
            .reshape(BPC, KC, 128, N).transpose(0, 2, 1, 3)
        ).astype(np.float16)  # [BPC, 128, KC, N]
        maskT_c = np.ascontiguousarray(
            decaymask[g * HPC:(g + 1) * HPC].transpose(0, 2, 1)
            .reshape(HPC, TC, 128, N).transpose(0, 2, 1, 3)
        ).astype(np.float16)  # [HPC, 128, kc, q]
        # fold 1/sqrt(HD) = 1/8 (exact) into wq/bq
        def packw(w):
            return np.ascontiguousarray(
                w.reshape(KC, 128, HPC * HD).transpose(1, 0, 2))
        wqT_c = packw(wq[rows, :].T * np.float32(0.125)).astype(np.float16)
        wkT_c = packw(wk[rows, :].T).astype(np.float16)
        wvT_c = packw(wv[rows, :].T).astype(np.float16)
        woT_c = np.ascontiguousarray(wo[:, rows].T).astype(np.float16)
        bq_c = np.ascontiguousarray(
            (bq[rows] * np.float32(0.125)).reshape(NSTACK, 128).T)
        bk_c = np.ascontiguousarray(bk[rows].reshape(NSTACK, 128).T)
        in_maps.append({
            "xT": xT_c,
            "maskT": maskT_c,
            "wqT": wqT_c,
            "wkT": wkT_c,
            "wvT": wvT_c,
            "woT": woT_c,
            "bq": bq_c,
            "bk": bk_c,
        })

    res = run_bass_kernel_spmd(nc, in_maps, list(range(NCORES)))
    LAST_RESULTS = res

    out = np.zeros((B, N, DIM), dtype=np.float32)
    for c in range(NCORES):
        g, p = c // 2, c % 2
        out[p * BPC:(p + 1) * BPC] += res.results[c]["outp"].astype(np.float32)
    out += (bo + bv @ wo.T)[None, None, :]
    return out
